# revision 2
# baseline (speedup 1.0000x reference)
"""Trainium2 Bass kernel for GQA attention layer (RoPE + causal + GQA 32q/8kv).

Self-contained: hardcodes shapes from the problem spec.
  hidden_states [2, 2048, 4096] f32, positions [2, 2048] i32,
  Wq [4096, 4096], Wk [1024, 4096], Wv [1024, 4096], Wo [4096, 4096]  (all f32)
Sharding: tensor-parallel over heads across 8 cores. Core c gets kv head c and
q heads 4c..4c+3. Each core computes its partial Wo output; host sums partials.

v2 structure: scores stay [k,q]; PV computes ctxT = v.T @ probs directly
(N=512, v stationary reused over heads); softmax denominator via all-ones
matmul that lands pre-broadcast in PSUM; Wo matmuls interleaved into the next
chunk's attention inner loop so the PE never idles while scalar does exp.
"""

import math
import os
import sys
import types

import numpy as np
import ml_dtypes

BF16NP = ml_dtypes.bfloat16

# ---- problem constants (hardcoded per spec) ----
P = 128
B = 2
S = 2048            # tokens per batch
HID = 4096
NH, NKV, HD = 32, 8, 128
NCORES = 8
HPC = NH // NCORES  # q heads per core (4)
T = B * S
SCALE = 1.0 / math.sqrt(HD)
ROPE_BASE = 10000.0

LAST = {}           # exec_time_ns etc from the most recent run


def _install_ntff_hook():
    """Register the axon NTFF profiling hook (image's antenv lacks axon_hooks)."""
    if "antenv.axon_hooks" in sys.modules:
        return
    try:
        import antenv
        mod = types.ModuleType("antenv.axon_hooks")
        _box = [None]
        mod.set_axon_ntff_profile_hook = lambda h: _box.__setitem__(0, h)
        mod.get_axon_ntff_profile_hook = lambda: _box[0]
        sys.modules["antenv.axon_hooks"] = mod
        antenv.axon_hooks = mod
        from trn_agent_boot.trn_boot import _ntff_profile_via_ctypes
        mod.set_axon_ntff_profile_hook(
            _ntff_profile_via_ctypes("/opt/axon/libaxon_pjrt.so")
        )
    except Exception:
        pass


def build_graph(S_=S, HID_=HID, CH=512, QC=512):
    import concourse.bacc as bacc
    import concourse.mybir as mybir
    import concourse.tile as tile
    from contextlib import ExitStack

    BF = mybir.dt.bfloat16
    F32 = mybir.dt.float32
    Exp = mybir.ActivationFunctionType.Exp

    NKK = HID_ // P          # contraction tiles over hidden (32)
    NCH = S_ // CH           # proj chunks per batch (4)
    NQC = S_ // QC           # attention q chunks per batch (4)
    NST = QC // P            # q subtiles per chunk (4)
    NKT = S_ // P            # k tiles per batch (16)
    NVS = CH // P            # v row-subtiles per proj chunk (4)
    HOC = HID_ // 512        # output column chunks (8)
    OCW = 512                # output chunk width
    NHG = HPC // 2           # head-pair passes per q chunk (2)

    nc = bacc.Bacc(None)
    xT_h = nc.declare_dram_parameter("xT", [HID_, B * S_], BF, isOutput=False)
    wq_h = nc.declare_dram_parameter("wqT", [HID_, HPC * HD], BF, isOutput=False)
    wk_h = nc.declare_dram_parameter("wkT", [HID_, HD], BF, isOutput=False)
    wv_h = nc.declare_dram_parameter("wvT", [HID_, HD], BF, isOutput=False)
    wo_h = nc.declare_dram_parameter("woT", [HPC * HD, HID_], BF, isOutput=False)
    cos_h = nc.declare_dram_parameter("cos2", [P, B * S_], BF, isOutput=False)
    sin_h = nc.declare_dram_parameter("sin2", [P, B * S_], BF, isOutput=False)
    mneg_h = nc.declare_dram_parameter("mneg", [P, P], F32, isOutput=False)
    iden_h = nc.declare_dram_parameter("iden", [P, P], BF, isOutput=False)
    out_h = nc.declare_dram_parameter("out", [B * S_, HID_], BF, isOutput=True)

    xT_r = xT_h[:, :].rearrange("(ko ki) s -> ki ko s", ki=P)
    wq_r = wq_h[:, :].rearrange("(ko ki) d -> ki ko d", ki=P)
    wk_r = wk_h[:, :].rearrange("(ko ki) d -> ki ko d", ki=P)
    wv_r = wv_h[:, :].rearrange("(ko ki) d -> ki ko d", ki=P)
    wo_r = wo_h[:, :].rearrange("(oo oi) h -> oi oo h", oi=P)

    with tile.TileContext(nc) as tc, ExitStack() as ctx:
        wpool = ctx.enter_context(tc.tile_pool(name="wpool", bufs=1))
        qpool = ctx.enter_context(tc.tile_pool(name="qpool", bufs=1))
        kpool = ctx.enter_context(tc.tile_pool(name="kpool", bufs=1))
        vpool = ctx.enter_context(tc.tile_pool(name="vpool", bufs=1))
        xpool = ctx.enter_context(tc.tile_pool(name="xpool", bufs=3))
        cspool = ctx.enter_context(tc.tile_pool(name="cspool", bufs=2))
        rpool = ctx.enter_context(tc.tile_pool(name="rpool", bufs=3))
        vtpool = ctx.enter_context(tc.tile_pool(name="vtpool", bufs=2))
        pbpool = ctx.enter_context(tc.tile_pool(name="pbpool", bufs=4))
        dipool = ctx.enter_context(tc.tile_pool(name="dipool", bufs=4))
        ctpool = ctx.enter_context(tc.tile_pool(name="ctpool", bufs=2))
        opool = ctx.enter_context(tc.tile_pool(name="opool", bufs=3))

        psS = ctx.enter_context(tc.tile_pool(name="psS", bufs=2, space="PSUM"))
        psC = ctx.enter_context(tc.tile_pool(name="psC", bufs=2, space="PSUM"))
        psD = ctx.enter_context(tc.tile_pool(name="psD", bufs=2, space="PSUM"))
        psX = ctx.enter_context(tc.tile_pool(name="psX", bufs=2, space="PSUM"))

        # --- persistent weights / tables ---
        # Order DMAs so the first K-proj matmuls can start ASAP:
        # wk, then x chunk 0 + cos/sin come per-chunk below, then wv/wq.
        wk_sb = wpool.tile([P, NKK, HD], BF)
        nc.sync.dma_start(out=wk_sb, in_=wk_r)
        wv_sb = wpool.tile([P, NKK, HD], BF)
        wq_sb = wpool.tile([P, NKK, HPC * HD], BF)
        wo_sb = wpool.tile([P, HPC, HID_], BF)
        mneg_sb = wpool.tile([P, P], F32)
        nc.sync.dma_start(out=mneg_sb, in_=mneg_h[:, :])
        iden_sb = wpool.tile([P, P], BF)
        nc.sync.dma_start(out=iden_sb, in_=iden_h[:, :])
        ones_sb = wpool.tile([P, P], BF)
        nc.vector.memset(ones_sb, 1.0)

        # Wo-step interleave queue: closures that emit a bit of the previous
        # chunk's Wo GEMM (4 matmuls + a psum drain copy [+ dma]).
        pending = []

        def drain_wo(n):
            for _ in range(min(n, len(pending))):
                pending.pop(0)()

        def make_wo_steps(b, qc, ct):
            """Build the list of Wo steps for q-chunk qc of batch b."""
            steps = []
            box = {}

            def step(hc, si4):
                def run():
                    if si4 == 0:
                        box[hc] = opool.tile([P, NST, OCW], BF, name="ob", tag="ob")
                    ob = box[hc]
                    po = psX.tile([P, OCW], F32, name="po", tag="px")
                    for ot in range(HPC):
                        nc.tensor.matmul(
                            po,
                            lhsT=ct[:, ot, si4 * P:(si4 + 1) * P],
                            rhs=wo_sb[:, ot, hc * OCW:(hc + 1) * OCW],
                            start=(ot == 0), stop=(ot == HPC - 1),
                        )
                    if (si4 + hc) % 2 == 0:
                        nc.vector.tensor_copy(out=ob[:, si4, :], in_=po)
                    else:
                        nc.scalar.copy(out=ob[:, si4, :], in_=po)
                    if si4 == NST - 1:
                        orows = out_h[b * S_ + qc * QC: b * S_ + (qc + 1) * QC,
                                      hc * OCW:(hc + 1) * OCW]
                        nc.sync.dma_start(
                            out=orows.rearrange("(si p) h -> p si h", p=P),
                            in_=ob,
                        )
                return run

            for hc in range(HOC):
                for si4 in range(NST):
                    steps.append(step(hc, si4))
            return steps

        def rope(ps, dst, cs, sn):
            """Neox RoPE on [128 d, n] tile: rows 0:64 = first half of head dim."""
            qf = rpool.tile([P, CH], BF, tag="qf")
            nc.vector.tensor_copy(out=qf, in_=ps)
            qs = rpool.tile([P, CH], BF, tag="qs")
            nc.gpsimd.dma_start(out=qs[0:64, :], in_=qf[64:128, :])
            nc.gpsimd.dma_start(out=qs[64:128, :], in_=qf[0:64, :])
            nc.vector.tensor_mul(out=qf, in0=qf, in1=cs)
            nc.vector.tensor_mul(out=qs, in0=qs, in1=sn)
            nc.vector.tensor_add(out=dst, in0=qf, in1=qs)

        for b in range(B):
            # ---------- phase P: projections + RoPE ----------
            qT = qpool.tile([P, HPC, S_], BF)
            kT = kpool.tile([P, S_], BF)
            v = vpool.tile([P, NKT, P], BF)
            NKH = NKK // 2
            for t in range(NCH):
                c0 = b * S_ + t * CH
                c1 = c0 + CH
                xta = xpool.tile([P, NKH, CH], BF, tag="x")
                nc.sync.dma_start(out=xta, in_=xT_r[:, 0:NKH, c0:c1])
                xtb = xpool.tile([P, NKH, CH], BF, tag="x")
                nc.sync.dma_start(out=xtb, in_=xT_r[:, NKH:NKK, c0:c1])
                cs = cspool.tile([P, CH], BF, tag="cos")
                nc.sync.dma_start(out=cs, in_=cos_h[:, c0:c1])
                sn = cspool.tile([P, CH], BF, tag="sin")
                nc.sync.dma_start(out=sn, in_=sin_h[:, c0:c1])
                if b == 0 and t == 0:
                    # remaining weights, after the critical chunk-0 tensors
                    nc.sync.dma_start(out=wv_sb, in_=wv_r)
                    for _wi in range(4):
                        _lo, _hi = _wi * NKK // 4, (_wi + 1) * NKK // 4
                        nc.sync.dma_start(
                            out=wq_sb[:, _lo:_hi, :], in_=wq_r[:, _lo:_hi, :]
                        )

                def xt(kk):
                    return xta[:, kk, :] if kk < NKH else xtb[:, kk - NKH, :]

                # K projection first (smallest weight dep)
                ps = psX.tile([P, CH], F32, tag="px")
                for kk in range(NKK):
                    nc.tensor.matmul(
                        ps, lhsT=wk_sb[:, kk, :], rhs=xt(kk),
                        start=(kk == 0), stop=(kk == NKK - 1),
                    )
                rope(ps, kT[:, t * CH:t * CH + CH], cs, sn)
                drain_wo(1)
                # V projection in vT orientation, then PE-transpose to [s, d]
                pv = psX.tile([P, CH], F32, tag="px")
                for kk in range(NKK):
                    nc.tensor.matmul(
                        pv, lhsT=wv_sb[:, kk, :], rhs=xt(kk),
                        start=(kk == 0), stop=(kk == NKK - 1),
                    )
                vt = vtpool.tile([P, CH], BF, tag="vt")
                nc.vector.tensor_copy(out=vt, in_=pv)
                for ss in range(NVS):
                    pq = psS.tile([P, P], BF, tag="s")
                    nc.tensor.transpose(pq, vt[:, ss * P:(ss + 1) * P], iden_sb)
                    nc.scalar.copy(out=v[:, t * NVS + ss, :], in_=pq)
                drain_wo(1)
                for g in range(HPC):
                    ps = psX.tile([P, CH], F32, tag="px")
                    for kk in range(NKK):
                        nc.tensor.matmul(
                            ps,
                            lhsT=wq_sb[:, kk, g * HD:(g + 1) * HD],
                            rhs=xt(kk),
                            start=(kk == 0), stop=(kk == NKK - 1),
                        )
                    rope(ps, qT[:, g, t * CH:t * CH + CH], cs, sn)
                    drain_wo(1)
                if b == 0 and t == 2:
                    for _wi in range(4):
                        _lo, _hi = _wi * HID_ // 4, (_wi + 1) * HID_ // 4
                        nc.sync.dma_start(
                            out=wo_sb[:, :, _lo:_hi], in_=wo_r[:, :, _lo:_hi]
                        )

            # ---------- phase A: attention ----------
            for qc in range(NQC):
                nkt = (qc + 1) * NST
                ct = ctpool.tile([P, HPC, QC], BF, name="ct", tag="ct")
                # distribute leftover Wo work evenly over this chunk's iters
                niter = NHG * (nkt + 1)
                quota = len(pending) / max(niter, 1)
                acc = [0.0]

                def drain_quota():
                    acc[0] += quota
                    k = int(acc[0])
                    if k:
                        acc[0] -= k
                        drain_wo(k)

                for hg in range(NHG):
                    h0 = hg * 2
                    pcs = {}
                    pdb = {}
                    pbs = {}
                    for kt in range(nkt + 1):
                        if kt < nkt:
                            d = kt - qc * NST  # diag subtile index if >= 0
                            pss = {}
                            for hh in range(2):
                                h = h0 + hh
                                pt = psS.tile([P, QC], F32, name="pss", tag="s")
                                nc.tensor.matmul(
                                    pt,
                                    lhsT=kT[:, kt * P:(kt + 1) * P],
                                    rhs=qT[:, h, qc * QC:(qc + 1) * QC],
                                    start=True, stop=True,
                                )
                                pss[hh] = pt
                            if d >= 0:
                                for hh in range(2):
                                    nc.vector.tensor_add(
                                        out=pss[hh][:, d * P:(d + 1) * P],
                                        in0=pss[hh][:, d * P:(d + 1) * P],
                                        in1=mneg_sb,
                                    )
                            for hh in range(2):
                                pb = pbpool.tile([P, QC], BF, name="pb", tag="pb")
                                lo = d * P if d > 0 else 0
                                nc.scalar.activation(
                                    out=pb[:, lo:QC], in_=pss[hh][:, lo:QC],
                                    func=Exp, scale=SCALE,
                                )
                                if lo:
                                    nc.gpsimd.memset(pb[:, 0:lo], 0.0)
                                pbs[(kt, hh)] = pb
                        if kt > 0:
                            kp = kt - 1
                            first, last = (kp == 0), (kp == nkt - 1)
                            for hh in range(2):
                                pb = pbs.pop((kp, hh))
                                if first:
                                    pcs[hh] = psC.tile(
                                        [P, QC], F32, name="pctx", tag="ctx"
                                    )
                                    pdb[hh] = psD.tile(
                                        [P, QC], F32, name="pdbc", tag="dbc"
                                    )
                                nc.tensor.matmul(
                                    pcs[hh],
                                    lhsT=v[:, kp, :],
                                    rhs=pb,
                                    start=first, stop=last,
                                )
                                nc.tensor.matmul(
                                    pdb[hh],
                                    lhsT=ones_sb,
                                    rhs=pb,
                                    start=first, stop=last,
                                )
                        drain_quota()
                    # normalize this head pair: ctxT = pcs * (1/denominator)
                    for hh in range(2):
                        dinv = dipool.tile([P, QC], F32, name="dinv", tag="di")
                        nc.vector.reciprocal(out=dinv, in_=pdb[hh])
                        nc.vector.tensor_mul(
                            out=ct[:, h0 + hh, :], in0=pcs[hh], in1=dinv
                        )
                drain_wo(len(pending))  # anything left from previous chunk
                pending = make_wo_steps(b, qc, ct)

        drain_wo(len(pending))

    nc.compile()
    return nc


_CACHE = {}


def _get_graph():
    if "nc" not in _CACHE:
        _CACHE["nc"] = build_graph()
    return _CACHE["nc"]


def _host_prep(hidden_states, positions, Wq, Wk, Wv, Wo):
    """Transpose/cast/slice inputs per core. Returns list of 8 input dicts."""
    x2 = np.ascontiguousarray(hidden_states.reshape(T, HID).T).astype(BF16NP)

    pos = positions.astype(np.float32)                      # [B, S]
    half = HD // 2
    inv_freq = 1.0 / (ROPE_BASE ** (np.arange(half, dtype=np.float32) / half))
    ang = pos[:, :, None] * inv_freq[None, None, :]         # [B, S, 64]
    cos = np.cos(ang)
    sin = np.sin(ang)
    cosT = np.concatenate([cos[b].T for b in range(B)], axis=1)   # [64, T]
    sinT = np.concatenate([sin[b].T for b in range(B)], axis=1)
    cos2 = np.concatenate([cosT, cosT], axis=0).astype(BF16NP)    # [128, T]
    sin2 = np.concatenate([-sinT, sinT], axis=0).astype(BF16NP)

    r = np.arange(P)
    mneg = np.where(r[:, None] <= r[None, :], 0.0, -1e30).astype(np.float32)
    iden = np.eye(P, dtype=np.float32).astype(BF16NP)

    in_maps = []
    for c in range(NCORES):
        qs = slice(c * HPC * HD, (c + 1) * HPC * HD)
        ks = slice(c * HD, (c + 1) * HD)
        in_maps.append({
            "xT": x2,
            "wqT": np.ascontiguousarray(Wq[qs, :].T).astype(BF16NP),
            "wkT": np.ascontiguousarray(Wk[ks, :].T).astype(BF16NP),
            "wvT": np.ascontiguousarray(Wv[ks, :].T).astype(BF16NP),
            "woT": np.ascontiguousarray(Wo[:, qs].T).astype(BF16NP),
            "cos2": cos2,
            "sin2": sin2,
            "mneg": mneg,
            "iden": iden,
        })
    return in_maps


def kernel(hidden_states, positions, Wq, Wk, Wv, Wo):
    from concourse.bass_utils import run_bass_kernel_spmd

    trace = bool(os.environ.get("CLAUDE_KERNEL_TRACE"))
    if trace:
        _install_ntff_hook()

    nc = _get_graph()
    in_maps = _host_prep(
        np.asarray(hidden_states), np.asarray(positions),
        np.asarray(Wq), np.asarray(Wk), np.asarray(Wv), np.asarray(Wo),
    )
    res = run_bass_kernel_spmd(
        nc, in_maps, core_ids=list(range(NCORES)), trace=trace,
    )
    LAST["exec_time_ns"] = res.exec_time_ns
    LAST["profile_json"] = res.profile_json
    if res.instructions_and_trace is not None:
        LAST["trace_path"] = res.instructions_and_trace[1]

    acc = np.zeros((T, HID), np.float32)
    for c in range(NCORES):
        acc += res.results[c]["out"].astype(np.float32)
    return acc.reshape(B, S, HID)


# revision 9
# speedup vs baseline: 1.0137x; 1.0137x over previous
"""Trainium2 Bass kernel for GQA attention layer (RoPE + causal + GQA 32q/8kv).

Self-contained: hardcodes shapes from the problem spec.
  hidden_states [2, 2048, 4096] f32, positions [2, 2048] i32,
  Wq [4096, 4096], Wk [1024, 4096], Wv [1024, 4096], Wo [4096, 4096]  (all f32)
Sharding: tensor-parallel over heads across 8 cores. Core c gets kv head c and
q heads 4c..4c+3. Each core computes its partial Wo output; host sums partials.

v2 structure: scores stay [k,q]; PV computes ctxT = v.T @ probs directly
(N=512, v stationary reused over heads); softmax denominator via all-ones
matmul that lands pre-broadcast in PSUM; Wo matmuls interleaved into the next
chunk's attention inner loop so the PE never idles while scalar does exp.
"""

import math
import os
import sys
import types

import numpy as np
import ml_dtypes

BF16NP = ml_dtypes.bfloat16

# ---- problem constants (hardcoded per spec) ----
P = 128
B = 2
S = 2048            # tokens per batch
HID = 4096
NH, NKV, HD = 32, 8, 128
NCORES = 8
HPC = NH // NCORES  # q heads per core (4)
T = B * S
SCALE = 1.0 / math.sqrt(HD)
ROPE_BASE = 10000.0

LAST = {}           # exec_time_ns etc from the most recent run


def _install_ntff_hook():
    """Register the axon NTFF profiling hook (image's antenv lacks axon_hooks)."""
    if "antenv.axon_hooks" in sys.modules:
        return
    try:
        import antenv
        mod = types.ModuleType("antenv.axon_hooks")
        _box = [None]
        mod.set_axon_ntff_profile_hook = lambda h: _box.__setitem__(0, h)
        mod.get_axon_ntff_profile_hook = lambda: _box[0]
        sys.modules["antenv.axon_hooks"] = mod
        antenv.axon_hooks = mod
        from trn_agent_boot.trn_boot import _ntff_profile_via_ctypes
        mod.set_axon_ntff_profile_hook(
            _ntff_profile_via_ctypes("/opt/axon/libaxon_pjrt.so")
        )
    except Exception:
        pass


def build_graph(S_=S, HID_=HID, CH=512, QC=512):
    import concourse.bacc as bacc
    import concourse.mybir as mybir
    import concourse.tile as tile
    from contextlib import ExitStack

    BF = mybir.dt.bfloat16
    F32 = mybir.dt.float32
    Exp = mybir.ActivationFunctionType.Exp

    NKK = HID_ // P          # contraction tiles over hidden (32)
    NCH = S_ // CH           # proj chunks per batch (4)
    NQC = S_ // QC           # attention q chunks per batch (4)
    NST = QC // P            # q subtiles per chunk (4)
    NKT = S_ // P            # k tiles per batch (16)
    NVS = CH // P            # v row-subtiles per proj chunk (4)
    HOC = HID_ // 512        # output column chunks (8)
    OCW = 512                # output chunk width
    NHG = HPC // 2           # head-pair passes per q chunk (2)

    nc = bacc.Bacc(None)
    xT_h = nc.declare_dram_parameter("xT", [HID_, B * S_], BF, isOutput=False)
    wq_h = nc.declare_dram_parameter("wqT", [HID_, HPC * HD], BF, isOutput=False)
    wk_h = nc.declare_dram_parameter("wkT", [HID_, HD], BF, isOutput=False)
    wv_h = nc.declare_dram_parameter("wvT", [HID_, HD], BF, isOutput=False)
    wo_h = nc.declare_dram_parameter("woT", [HPC * HD, HID_], BF, isOutput=False)
    cos_h = nc.declare_dram_parameter("cos2", [P, B * S_], BF, isOutput=False)
    sin_h = nc.declare_dram_parameter("sin2", [P, B * S_], BF, isOutput=False)
    tri_h = nc.declare_dram_parameter("tri", [P, P], BF, isOutput=False)
    iden_h = nc.declare_dram_parameter("iden", [P, P], BF, isOutput=False)
    out_h = nc.declare_dram_parameter("out", [B * S_, HID_], BF, isOutput=True)

    xT_r = xT_h[:, :].rearrange("(ko ki) s -> ki ko s", ki=P)
    wq_r = wq_h[:, :].rearrange("(ko ki) d -> ki ko d", ki=P)
    wk_r = wk_h[:, :].rearrange("(ko ki) d -> ki ko d", ki=P)
    wv_r = wv_h[:, :].rearrange("(ko ki) d -> ki ko d", ki=P)
    wo_r = wo_h[:, :].rearrange("(oo oi) h -> oi oo h", oi=P)

    with tile.TileContext(nc) as tc, ExitStack() as ctx:
        wpool = ctx.enter_context(tc.tile_pool(name="wpool", bufs=1))
        qpool = ctx.enter_context(tc.tile_pool(name="qpool", bufs=1))
        kpool = ctx.enter_context(tc.tile_pool(name="kpool", bufs=1))
        vpool = ctx.enter_context(tc.tile_pool(name="vpool", bufs=1))
        xpool = ctx.enter_context(tc.tile_pool(name="xpool", bufs=3))
        cspool = ctx.enter_context(tc.tile_pool(name="cspool", bufs=2))
        rpool = ctx.enter_context(tc.tile_pool(name="rpool", bufs=3))
        vtpool = ctx.enter_context(tc.tile_pool(name="vtpool", bufs=2))
        pbpool = ctx.enter_context(tc.tile_pool(name="pbpool", bufs=6))
        dipool = ctx.enter_context(tc.tile_pool(name="dipool", bufs=4))
        ctpool = ctx.enter_context(tc.tile_pool(name="ctpool", bufs=2))
        opool = ctx.enter_context(tc.tile_pool(name="opool", bufs=3))

        psS = ctx.enter_context(tc.tile_pool(name="psS", bufs=2, space="PSUM"))
        psC = ctx.enter_context(tc.tile_pool(name="psC", bufs=2, space="PSUM"))
        psD = ctx.enter_context(tc.tile_pool(name="psD", bufs=2, space="PSUM"))
        psX = ctx.enter_context(tc.tile_pool(name="psX", bufs=2, space="PSUM"))

        # --- persistent weights / tables ---
        # Order DMAs so the first K-proj matmuls can start ASAP:
        # wk, then x chunk 0 + cos/sin come per-chunk below, then wv/wq.
        wk_sb = wpool.tile([P, NKK, HD], BF)
        nc.sync.dma_start(out=wk_sb, in_=wk_r)
        wv_sb = wpool.tile([P, NKK, HD], BF)
        wq_sb = wpool.tile([P, NKK, HPC * HD], BF)
        wo_sb = wpool.tile([P, HPC, HID_], BF)
        tri_sb = wpool.tile([P, P], BF)
        nc.sync.dma_start(out=tri_sb, in_=tri_h[:, :])
        iden_sb = wpool.tile([P, P], BF)
        nc.sync.dma_start(out=iden_sb, in_=iden_h[:, :])
        ones_sb = wpool.tile([P, P], BF)
        nc.vector.memset(ones_sb, 1.0)

        # Wo-step interleave queue: closures that emit a bit of the previous
        # chunk's Wo GEMM (4 matmuls + a psum drain copy [+ dma]).
        pending = []

        def drain_wo(n):
            for _ in range(min(n, len(pending))):
                pending.pop(0)()

        def make_wo_steps(b, qc, ct):
            """Build the list of Wo steps for q-chunk qc of batch b."""
            steps = []
            box = {}

            def step(hc, si4):
                def run():
                    if si4 == 0:
                        box[hc] = opool.tile([P, NST, OCW], BF, name="ob", tag="ob")
                    ob = box[hc]
                    po = psX.tile([P, OCW], F32, name="po", tag="px")
                    for ot in range(HPC):
                        nc.tensor.matmul(
                            po,
                            lhsT=ct[:, ot, si4 * P:(si4 + 1) * P],
                            rhs=wo_sb[:, ot, hc * OCW:(hc + 1) * OCW],
                            start=(ot == 0), stop=(ot == HPC - 1),
                        )
                    if (si4 + hc) % 2 == 0:
                        nc.vector.tensor_copy(out=ob[:, si4, :], in_=po)
                    else:
                        nc.scalar.copy(out=ob[:, si4, :], in_=po)
                    if si4 == NST - 1:
                        orows = out_h[b * S_ + qc * QC: b * S_ + (qc + 1) * QC,
                                      hc * OCW:(hc + 1) * OCW]
                        nc.sync.dma_start(
                            out=orows.rearrange("(si p) h -> p si h", p=P),
                            in_=ob,
                        )
                return run

            for hc in range(HOC):
                for si4 in range(NST):
                    steps.append(step(hc, si4))
            return steps

        def rope(ps, dst, cs, sn):
            """Neox RoPE on [128 d, n] tile: rows 0:64 = first half of head dim."""
            qf = rpool.tile([P, CH], BF, tag="qf")
            nc.vector.tensor_copy(out=qf, in_=ps)
            qs = rpool.tile([P, CH], BF, tag="qs")
            nc.gpsimd.dma_start(out=qs[0:64, :], in_=qf[64:128, :])
            nc.gpsimd.dma_start(out=qs[64:128, :], in_=qf[0:64, :])
            nc.vector.tensor_mul(out=qf, in0=qf, in1=cs)
            nc.vector.tensor_mul(out=qs, in0=qs, in1=sn)
            nc.vector.tensor_add(out=dst, in0=qf, in1=qs)

        for b in range(B):
            # ---------- phase P: projections + RoPE ----------
            qT = qpool.tile([P, HPC, S_], BF)
            kT = kpool.tile([P, S_], BF)
            v = vpool.tile([P, NKT, P], BF)
            NKH = NKK // 2
            for t in range(NCH):
                c0 = b * S_ + t * CH
                c1 = c0 + CH
                xta = xpool.tile([P, NKH, CH], BF, tag="x")
                nc.sync.dma_start(out=xta, in_=xT_r[:, 0:NKH, c0:c1])
                xtb = xpool.tile([P, NKH, CH], BF, tag="x")
                nc.sync.dma_start(out=xtb, in_=xT_r[:, NKH:NKK, c0:c1])
                cs = cspool.tile([P, CH], BF, tag="cos")
                nc.sync.dma_start(out=cs, in_=cos_h[:, c0:c1])
                sn = cspool.tile([P, CH], BF, tag="sin")
                nc.sync.dma_start(out=sn, in_=sin_h[:, c0:c1])
                if b == 0 and t == 0:
                    # remaining weights, after the critical chunk-0 tensors
                    nc.sync.dma_start(out=wv_sb, in_=wv_r)
                    for _wi in range(4):
                        _lo, _hi = _wi * NKK // 4, (_wi + 1) * NKK // 4
                        nc.sync.dma_start(
                            out=wq_sb[:, _lo:_hi, :], in_=wq_r[:, _lo:_hi, :]
                        )

                def xt(kk):
                    return xta[:, kk, :] if kk < NKH else xtb[:, kk - NKH, :]

                # K projection first (smallest weight dep)
                ps = psX.tile([P, CH], F32, tag="px")
                for kk in range(NKK):
                    nc.tensor.matmul(
                        ps, lhsT=wk_sb[:, kk, :], rhs=xt(kk),
                        start=(kk == 0), stop=(kk == NKK - 1),
                    )
                rope(ps, kT[:, t * CH:t * CH + CH], cs, sn)
                drain_wo(1)
                # V projection in vT orientation, then PE-transpose to [s, d]
                pv = psX.tile([P, CH], F32, tag="px")
                for kk in range(NKK):
                    nc.tensor.matmul(
                        pv, lhsT=wv_sb[:, kk, :], rhs=xt(kk),
                        start=(kk == 0), stop=(kk == NKK - 1),
                    )
                vt = vtpool.tile([P, CH], BF, tag="vt")
                nc.vector.tensor_copy(out=vt, in_=pv)
                for ss in range(NVS):
                    pq = psS.tile([P, P], BF, tag="s")
                    nc.tensor.transpose(pq, vt[:, ss * P:(ss + 1) * P], iden_sb)
                    nc.scalar.copy(out=v[:, t * NVS + ss, :], in_=pq)
                drain_wo(1)
                for g in range(HPC):
                    ps = psX.tile([P, CH], F32, tag="px")
                    for kk in range(NKK):
                        nc.tensor.matmul(
                            ps,
                            lhsT=wq_sb[:, kk, g * HD:(g + 1) * HD],
                            rhs=xt(kk),
                            start=(kk == 0), stop=(kk == NKK - 1),
                        )
                    rope(ps, qT[:, g, t * CH:t * CH + CH], cs, sn)
                    drain_wo(1)
                if b == 0 and t == 2:
                    for _wi in range(4):
                        _lo, _hi = _wi * HID_ // 4, (_wi + 1) * HID_ // 4
                        nc.sync.dma_start(
                            out=wo_sb[:, :, _lo:_hi], in_=wo_r[:, :, _lo:_hi]
                        )

            # ---------- phase A: attention ----------
            for qc in range(NQC):
                nkt = (qc + 1) * NST
                ct = ctpool.tile([P, HPC, QC], BF, name="ct", tag="ct")
                # distribute leftover Wo work evenly over this chunk's iters
                niter = NHG * (nkt + 1)
                quota = len(pending) / max(niter, 1)
                acc = [0.0]

                def drain_quota():
                    acc[0] += quota
                    k = int(acc[0])
                    if k:
                        acc[0] -= k
                        drain_wo(k)

                for hg in range(NHG):
                    h0 = hg * 2
                    pcs = {}
                    pdb = {}
                    pbs = {}
                    ph1 = {}
                    for kt in range(nkt + 1):
                        if kt < nkt:
                            d = kt - qc * NST  # diag subtile index if >= 0
                            lo = d * P if d > 0 else 0
                            pss = {}
                            for hh in range(2):
                                h = h0 + hh
                                pt = psS.tile([P, QC], F32, name="pss", tag="s")
                                nc.tensor.matmul(
                                    pt[:, lo:QC],
                                    lhsT=kT[:, kt * P:(kt + 1) * P],
                                    rhs=qT[:, h, qc * QC + lo:(qc + 1) * QC],
                                    start=True, stop=True,
                                )
                                pss[hh] = pt
                            for hh in range(2):
                                pb = pbpool.tile([P, QC], BF, name="pb", tag="pb")
                                nc.scalar.activation(
                                    out=pb[:, lo:QC], in_=pss[hh][:, lo:QC],
                                    func=Exp, scale=SCALE,
                                )
                                if d >= 0:
                                    # zero strictly-upper part of the diagonal
                                    # 128-subtile (keep k row <= q col)
                                    nc.gpsimd.tensor_mul(
                                        out=pb[:, d * P:(d + 1) * P],
                                        in0=pb[:, d * P:(d + 1) * P],
                                        in1=tri_sb,
                                    )
                                if lo:
                                    nc.gpsimd.memset(pb[:, 0:lo], 0.0)
                                pbs[(kt, hh)] = pb
                        if kt > 0:
                            kp = kt - 1
                            first, last = (kp == 0), (kp == nkt - 1)
                            dp = kp - qc * NST
                            lop = dp * P if dp > 0 else 0
                            for hh in range(2):
                                pb = pbs[(kp, hh)]
                                if first:
                                    pcs[hh] = psC.tile(
                                        [P, QC], F32, name="pctx", tag="ctx"
                                    )
                                    pdb[hh] = psD.tile(
                                        [P, QC], F32, name="pdbc", tag="dbc"
                                    )
                                nc.tensor.matmul(
                                    pcs[hh][:, lop:QC],
                                    lhsT=v[:, kp, :],
                                    rhs=pb[:, lop:QC],
                                    start=first, stop=last,
                                )
                                # denominator: sum groups of 4 kt on gpsimd,
                                # one all-ones matmul per group (broadcast D)
                                if kp % 4 == 1:
                                    s2 = pbpool.tile(
                                        [P, QC], BF, name="s2", tag="s2", bufs=4
                                    )
                                    nc.gpsimd.tensor_add(
                                        out=s2, in0=pbs.pop((kp - 1, hh)), in1=pb
                                    )
                                    del pbs[(kp, hh)]
                                    ph1[hh] = s2
                                elif kp % 4 == 3:
                                    s2 = pbpool.tile(
                                        [P, QC], BF, name="s2b", tag="s2b", bufs=2
                                    )
                                    nc.gpsimd.tensor_add(
                                        out=s2, in0=pbs.pop((kp - 1, hh)), in1=pb
                                    )
                                    del pbs[(kp, hh)]
                                    s4 = pbpool.tile(
                                        [P, QC], BF, name="s4", tag="s4", bufs=2
                                    )
                                    nc.gpsimd.tensor_add(
                                        out=s4, in0=ph1.pop(hh), in1=s2
                                    )
                                    nc.tensor.matmul(
                                        pdb[hh],
                                        lhsT=ones_sb,
                                        rhs=s4,
                                        start=(kp == 3), stop=last,
                                    )
                        drain_quota()
                    # normalize this head pair: ctxT = pcs * (1/denominator)
                    for hh in range(2):
                        dinv = dipool.tile([P, QC], F32, name="dinv", tag="di")
                        nc.vector.reciprocal_approx_fast(out=dinv, in_=pdb[hh])
                        nc.vector.tensor_mul(
                            out=ct[:, h0 + hh, :], in0=pcs[hh], in1=dinv
                        )
                drain_wo(len(pending))  # anything left from previous chunk
                pending = make_wo_steps(b, qc, ct)

        drain_wo(len(pending))

    nc.compile()
    return nc


_CACHE = {}


def _get_graph():
    if "nc" not in _CACHE:
        _CACHE["nc"] = build_graph()
    return _CACHE["nc"]


def _host_prep(hidden_states, positions, Wq, Wk, Wv, Wo):
    """Transpose/cast/slice inputs per core. Returns list of 8 input dicts."""
    x2 = np.ascontiguousarray(hidden_states.reshape(T, HID).T).astype(BF16NP)

    pos = positions.astype(np.float32)                      # [B, S]
    half = HD // 2
    inv_freq = 1.0 / (ROPE_BASE ** (np.arange(half, dtype=np.float32) / half))
    ang = pos[:, :, None] * inv_freq[None, None, :]         # [B, S, 64]
    cos = np.cos(ang)
    sin = np.sin(ang)
    cosT = np.concatenate([cos[b].T for b in range(B)], axis=1)   # [64, T]
    sinT = np.concatenate([sin[b].T for b in range(B)], axis=1)
    cos2 = np.concatenate([cosT, cosT], axis=0).astype(BF16NP)    # [128, T]
    sin2 = np.concatenate([-sinT, sinT], axis=0).astype(BF16NP)

    r = np.arange(P)
    tri = (r[:, None] <= r[None, :]).astype(np.float32).astype(BF16NP)
    iden = np.eye(P, dtype=np.float32).astype(BF16NP)

    in_maps = []
    for c in range(NCORES):
        qs = slice(c * HPC * HD, (c + 1) * HPC * HD)
        ks = slice(c * HD, (c + 1) * HD)
        in_maps.append({
            "xT": x2,
            "wqT": np.ascontiguousarray(Wq[qs, :].T).astype(BF16NP),
            "wkT": np.ascontiguousarray(Wk[ks, :].T).astype(BF16NP),
            "wvT": np.ascontiguousarray(Wv[ks, :].T).astype(BF16NP),
            "woT": np.ascontiguousarray(Wo[:, qs].T).astype(BF16NP),
            "cos2": cos2,
            "sin2": sin2,
            "tri": tri,
            "iden": iden,
        })
    return in_maps


def kernel(hidden_states, positions, Wq, Wk, Wv, Wo):
    from concourse.bass_utils import run_bass_kernel_spmd

    trace = bool(os.environ.get("CLAUDE_KERNEL_TRACE"))
    if trace:
        _install_ntff_hook()

    nc = _get_graph()
    in_maps = _host_prep(
        np.asarray(hidden_states), np.asarray(positions),
        np.asarray(Wq), np.asarray(Wk), np.asarray(Wv), np.asarray(Wo),
    )
    res = run_bass_kernel_spmd(
        nc, in_maps, core_ids=list(range(NCORES)), trace=trace,
    )
    LAST["exec_time_ns"] = res.exec_time_ns
    LAST["profile_json"] = res.profile_json
    if res.instructions_and_trace is not None:
        LAST["trace_path"] = res.instructions_and_trace[1]

    acc = np.zeros((T, HID), np.float32)
    for c in range(NCORES):
        acc += res.results[c]["out"].astype(np.float32)
    return acc.reshape(B, S, HID)


# revision 13
# speedup vs baseline: 1.1917x; 1.1756x over previous
"""Trainium2 Bass kernel for GQA attention layer (RoPE + causal + GQA 32q/8kv).

Self-contained: hardcodes shapes from the problem spec.
  hidden_states [2, 2048, 4096] f32, positions [2, 2048] i32,
  Wq [4096, 4096], Wk [1024, 4096], Wv [1024, 4096], Wo [4096, 4096]  (all f32)
Sharding: tensor-parallel over heads across 8 cores. Core c gets kv head c and
q heads 4c..4c+3. Each core computes its partial Wo output; host sums partials.

v2 structure: scores stay [k,q]; PV computes ctxT = v.T @ probs directly
(N=512, v stationary reused over heads); softmax denominator via all-ones
matmul that lands pre-broadcast in PSUM; Wo matmuls interleaved into the next
chunk's attention inner loop so the PE never idles while scalar does exp.
"""

import math
import os
import sys
import types

import numpy as np
import ml_dtypes

BF16NP = ml_dtypes.bfloat16

# ---- problem constants (hardcoded per spec) ----
P = 128
B = 2
S = 2048            # tokens per batch
HID = 4096
NH, NKV, HD = 32, 8, 128
NCORES = 8
HPC = NH // NCORES  # q heads per core (4)
T = B * S
SCALE = 1.0 / math.sqrt(HD)
ROPE_BASE = 10000.0

LAST = {}           # exec_time_ns etc from the most recent run


def _install_ntff_hook():
    """Register the axon NTFF profiling hook (image's antenv lacks axon_hooks)."""
    if "antenv.axon_hooks" in sys.modules:
        return
    try:
        import antenv
        mod = types.ModuleType("antenv.axon_hooks")
        _box = [None]
        mod.set_axon_ntff_profile_hook = lambda h: _box.__setitem__(0, h)
        mod.get_axon_ntff_profile_hook = lambda: _box[0]
        sys.modules["antenv.axon_hooks"] = mod
        antenv.axon_hooks = mod
        from trn_agent_boot.trn_boot import _ntff_profile_via_ctypes
        mod.set_axon_ntff_profile_hook(
            _ntff_profile_via_ctypes("/opt/axon/libaxon_pjrt.so")
        )
    except Exception:
        pass


def build_graph(S_=S, HID_=HID, CH=512, QC=512):
    import concourse.bacc as bacc
    import concourse.mybir as mybir
    import concourse.tile as tile
    from contextlib import ExitStack

    BF = mybir.dt.bfloat16
    F32 = mybir.dt.float32
    Exp = mybir.ActivationFunctionType.Exp

    NKK = HID_ // P          # contraction tiles over hidden (32)
    NCH = S_ // CH           # proj chunks per batch (4)
    NQC = S_ // QC           # attention q chunks per batch (4)
    NST = QC // P            # q subtiles per chunk (4)
    NKT = S_ // P            # k tiles per batch (16)
    NVS = CH // P            # v row-subtiles per proj chunk (4)
    HOC = HID_ // 512        # output column chunks (8)
    OCW = 512                # output chunk width
    NHG = HPC // 2           # head-pair passes per q chunk (2)

    nc = bacc.Bacc(None)
    xT_h = nc.declare_dram_parameter("xT", [HID_, B * S_], BF, isOutput=False)
    wq_h = nc.declare_dram_parameter("wqT", [HID_, HPC * HD], BF, isOutput=False)
    wk_h = nc.declare_dram_parameter("wkT", [HID_, HD], BF, isOutput=False)
    wv_h = nc.declare_dram_parameter("wvT", [HID_, HD], BF, isOutput=False)
    wo_h = nc.declare_dram_parameter("woT", [HPC * HD, HID_], BF, isOutput=False)
    cos_h = nc.declare_dram_parameter("cos2", [P, B * S_], BF, isOutput=False)
    sin_h = nc.declare_dram_parameter("sin2", [P, B * S_], BF, isOutput=False)
    mneg_h = nc.declare_dram_parameter("mneg", [P, P], F32, isOutput=False)
    iden_h = nc.declare_dram_parameter("iden", [P, P], BF, isOutput=False)
    out_h = nc.declare_dram_parameter("out", [B * S_, HID_], BF, isOutput=True)

    xT_r = xT_h[:, :].rearrange("(ko ki) s -> ki ko s", ki=P)
    wq_r = wq_h[:, :].rearrange("(ko ki) d -> ki ko d", ki=P)
    wk_r = wk_h[:, :].rearrange("(ko ki) d -> ki ko d", ki=P)
    wv_r = wv_h[:, :].rearrange("(ko ki) d -> ki ko d", ki=P)
    wo_r = wo_h[:, :].rearrange("(oo oi) h -> oi oo h", oi=P)

    with tile.TileContext(nc) as tc, ExitStack() as ctx:
        wpool = ctx.enter_context(tc.tile_pool(name="wpool", bufs=1))
        qpool = ctx.enter_context(tc.tile_pool(name="qpool", bufs=1))
        kpool = ctx.enter_context(tc.tile_pool(name="kpool", bufs=1))
        vpool = ctx.enter_context(tc.tile_pool(name="vpool", bufs=1))
        xpool = ctx.enter_context(tc.tile_pool(name="xpool", bufs=3))
        cspool = ctx.enter_context(tc.tile_pool(name="cspool", bufs=2))
        rpool = ctx.enter_context(tc.tile_pool(name="rpool", bufs=3))
        vtpool = ctx.enter_context(tc.tile_pool(name="vtpool", bufs=2))
        pbpool = ctx.enter_context(tc.tile_pool(name="pbpool", bufs=6))
        dipool = ctx.enter_context(tc.tile_pool(name="dipool", bufs=4))
        ctpool = ctx.enter_context(tc.tile_pool(name="ctpool", bufs=2))
        opool = ctx.enter_context(tc.tile_pool(name="opool", bufs=3))

        psS = ctx.enter_context(tc.tile_pool(name="psS", bufs=2, space="PSUM"))
        psC = ctx.enter_context(tc.tile_pool(name="psC", bufs=2, space="PSUM"))
        psD = ctx.enter_context(tc.tile_pool(name="psD", bufs=2, space="PSUM"))
        psX = ctx.enter_context(tc.tile_pool(name="psX", bufs=2, space="PSUM"))

        # --- persistent weights / tables ---
        # Order DMAs so the first K-proj matmuls can start ASAP:
        # wk, then x chunk 0 + cos/sin come per-chunk below, then wv/wq.
        wk_sb = wpool.tile([P, NKK, HD], BF)
        nc.sync.dma_start(out=wk_sb, in_=wk_r)
        wv_sb = wpool.tile([P, NKK, HD], BF)
        wq_sb = wpool.tile([P, NKK, HPC * HD], BF)
        wo_sb = wpool.tile([P, HPC, HID_], BF)
        mneg_sb = wpool.tile([P, P], F32)
        nc.sync.dma_start(out=mneg_sb, in_=mneg_h[:, :])
        iden_sb = wpool.tile([P, P], BF)
        nc.sync.dma_start(out=iden_sb, in_=iden_h[:, :])
        ones_sb = wpool.tile([P, P], BF)
        nc.vector.memset(ones_sb, 1.0)

        # Wo-step interleave queue: closures that emit a bit of the previous
        # chunk's Wo GEMM (4 matmuls + a psum drain copy [+ dma]).
        pending = []

        def drain_wo(n):
            for _ in range(min(n, len(pending))):
                pending.pop(0)()

        def make_wo_steps(b, qc, ct):
            """Build the list of Wo steps for q-chunk qc of batch b."""
            steps = []
            box = {}

            def step(hc, si4):
                def run():
                    if si4 == 0:
                        box[hc] = opool.tile([P, NST, OCW], BF, name="ob", tag="ob")
                    ob = box[hc]
                    po = psX.tile([P, OCW], F32, name="po", tag="px")
                    for ot in range(HPC):
                        nc.tensor.matmul(
                            po,
                            lhsT=ct[:, ot, si4 * P:(si4 + 1) * P],
                            rhs=wo_sb[:, ot, hc * OCW:(hc + 1) * OCW],
                            start=(ot == 0), stop=(ot == HPC - 1),
                        )
                    if (si4 + hc) % 2 == 0:
                        nc.vector.tensor_copy(out=ob[:, si4, :], in_=po)
                    else:
                        nc.scalar.copy(out=ob[:, si4, :], in_=po)
                    if si4 == NST - 1:
                        orows = out_h[b * S_ + qc * QC: b * S_ + (qc + 1) * QC,
                                      hc * OCW:(hc + 1) * OCW]
                        nc.sync.dma_start(
                            out=orows.rearrange("(si p) h -> p si h", p=P),
                            in_=ob,
                        )
                return run

            for hc in range(HOC):
                for si4 in range(NST):
                    steps.append(step(hc, si4))
            return steps

        def rope(ps, dst, cs, sn):
            """Neox RoPE on [128 d, n] tile: rows 0:64 = first half of head dim."""
            qf = rpool.tile([P, CH], BF, tag="qf")
            nc.vector.tensor_copy(out=qf, in_=ps)
            qs = rpool.tile([P, CH], BF, tag="qs")
            nc.gpsimd.dma_start(out=qs[0:64, :], in_=qf[64:128, :])
            nc.gpsimd.dma_start(out=qs[64:128, :], in_=qf[0:64, :])
            nc.vector.tensor_mul(out=qf, in0=qf, in1=cs)
            nc.vector.tensor_mul(out=qs, in0=qs, in1=sn)
            nc.vector.tensor_add(out=dst, in0=qf, in1=qs)

        for b in range(B):
            # ---------- phase P: projections + RoPE ----------
            qT = qpool.tile([P, HPC, S_], BF)
            kT = kpool.tile([P, S_], BF)
            v = vpool.tile([P, NKT, P], BF)
            NKH = NKK // 2
            for t in range(NCH):
                c0 = b * S_ + t * CH
                c1 = c0 + CH
                xta = xpool.tile([P, NKH, CH], BF, tag="x")
                nc.sync.dma_start(out=xta, in_=xT_r[:, 0:NKH, c0:c1])
                xtb = xpool.tile([P, NKH, CH], BF, tag="x")
                nc.sync.dma_start(out=xtb, in_=xT_r[:, NKH:NKK, c0:c1])
                cs = cspool.tile([P, CH], BF, tag="cos")
                nc.sync.dma_start(out=cs, in_=cos_h[:, c0:c1])
                sn = cspool.tile([P, CH], BF, tag="sin")
                nc.sync.dma_start(out=sn, in_=sin_h[:, c0:c1])
                if b == 0 and t == 0:
                    # remaining weights, after the critical chunk-0 tensors
                    nc.sync.dma_start(out=wv_sb, in_=wv_r)
                    for _wi in range(4):
                        _lo, _hi = _wi * NKK // 4, (_wi + 1) * NKK // 4
                        nc.sync.dma_start(
                            out=wq_sb[:, _lo:_hi, :], in_=wq_r[:, _lo:_hi, :]
                        )

                def xt(kk):
                    return xta[:, kk, :] if kk < NKH else xtb[:, kk - NKH, :]

                # K projection first (smallest weight dep)
                ps = psX.tile([P, CH], F32, tag="px")
                for kk in range(NKK):
                    nc.tensor.matmul(
                        ps, lhsT=wk_sb[:, kk, :], rhs=xt(kk),
                        start=(kk == 0), stop=(kk == NKK - 1),
                    )
                rope(ps, kT[:, t * CH:t * CH + CH], cs, sn)
                drain_wo(1)
                # V projection in vT orientation, then PE-transpose to [s, d]
                pv = psX.tile([P, CH], F32, tag="px")
                for kk in range(NKK):
                    nc.tensor.matmul(
                        pv, lhsT=wv_sb[:, kk, :], rhs=xt(kk),
                        start=(kk == 0), stop=(kk == NKK - 1),
                    )
                vt = vtpool.tile([P, CH], BF, tag="vt")
                nc.vector.tensor_copy(out=vt, in_=pv)
                for ss in range(NVS):
                    pq = psS.tile([P, P], BF, tag="s")
                    nc.tensor.transpose(pq, vt[:, ss * P:(ss + 1) * P], iden_sb)
                    nc.scalar.copy(out=v[:, t * NVS + ss, :], in_=pq)
                drain_wo(1)
                for g in range(HPC):
                    ps = psX.tile([P, CH], F32, tag="px")
                    for kk in range(NKK):
                        nc.tensor.matmul(
                            ps,
                            lhsT=wq_sb[:, kk, g * HD:(g + 1) * HD],
                            rhs=xt(kk),
                            start=(kk == 0), stop=(kk == NKK - 1),
                        )
                    rope(ps, qT[:, g, t * CH:t * CH + CH], cs, sn)
                    drain_wo(1)
                if b == 0 and t == 2:
                    for _wi in range(4):
                        _lo, _hi = _wi * HID_ // 4, (_wi + 1) * HID_ // 4
                        nc.sync.dma_start(
                            out=wo_sb[:, :, _lo:_hi], in_=wo_r[:, :, _lo:_hi]
                        )

            # ---------- phase A: attention ----------
            for qc in range(NQC):
                nkt = (qc + 1) * NST
                ct = ctpool.tile([P, HPC, QC], BF, name="ct", tag="ct")
                # distribute leftover Wo work evenly over this chunk's iters
                niter = NHG * (nkt + 1)
                quota = len(pending) / max(niter, 1)
                acc = [0.0]

                def drain_quota():
                    acc[0] += quota
                    k = int(acc[0])
                    if k:
                        acc[0] -= k
                        drain_wo(k)

                for hg in range(NHG):
                    h0 = hg * 2
                    pcs = {}
                    pdb = {}
                    pbs = {}
                    ph1 = {}
                    for kt in range(nkt + 1):
                        if kt < nkt:
                            d = kt - qc * NST  # diag subtile index if >= 0
                            lo = d * P if d > 0 else 0
                            pss = {}
                            for hh in range(2):
                                h = h0 + hh
                                pt = psS.tile([P, QC], F32, name="pss", tag="s")
                                nc.tensor.matmul(
                                    pt[:, lo:QC],
                                    lhsT=kT[:, kt * P:(kt + 1) * P],
                                    rhs=qT[:, h, qc * QC + lo:(qc + 1) * QC],
                                    start=True, stop=True,
                                )
                                pss[hh] = pt
                            if d >= 0:
                                # mask strictly-upper part of the diagonal
                                # 128-subtile (keep k row <= q col)
                                for hh in range(2):
                                    nc.vector.tensor_add(
                                        out=pss[hh][:, d * P:(d + 1) * P],
                                        in0=pss[hh][:, d * P:(d + 1) * P],
                                        in1=mneg_sb,
                                    )
                            for hh in range(2):
                                pb = pbpool.tile([P, QC], BF, name="pb", tag="pb")
                                nc.scalar.activation(
                                    out=pb[:, lo:QC], in_=pss[hh][:, lo:QC],
                                    func=Exp, scale=SCALE,
                                )
                                if lo:
                                    nc.gpsimd.memset(pb[:, 0:lo], 0.0)
                                pbs[(kt, hh)] = pb
                        if kt > 0:
                            kp = kt - 1
                            first, last = (kp == 0), (kp == nkt - 1)
                            dp = kp - qc * NST
                            lop = dp * P if dp > 0 else 0
                            for hh in range(2):
                                pb = pbs[(kp, hh)]
                                if first:
                                    pcs[hh] = psC.tile(
                                        [P, QC], F32, name="pctx", tag="ctx"
                                    )
                                    pdb[hh] = psD.tile(
                                        [P, QC], F32, name="pdbc", tag="dbc"
                                    )
                                nc.tensor.matmul(
                                    pcs[hh][:, lop:QC],
                                    lhsT=v[:, kp, :],
                                    rhs=pb[:, lop:QC],
                                    start=first, stop=last,
                                )
                                # denominator: sum groups of 4 kt on gpsimd,
                                # one all-ones matmul per group (broadcast D)
                                if kp % 4 == 1:
                                    s2 = pbpool.tile(
                                        [P, QC], BF, name="s2", tag="s2", bufs=4
                                    )
                                    nc.vector.tensor_add(
                                        out=s2, in0=pbs.pop((kp - 1, hh)), in1=pb
                                    )
                                    del pbs[(kp, hh)]
                                    ph1[hh] = s2
                                elif kp % 4 == 3:
                                    s2 = pbpool.tile(
                                        [P, QC], BF, name="s2b", tag="s2b", bufs=2
                                    )
                                    nc.vector.tensor_add(
                                        out=s2, in0=pbs.pop((kp - 1, hh)), in1=pb
                                    )
                                    del pbs[(kp, hh)]
                                    s4 = pbpool.tile(
                                        [P, QC], BF, name="s4", tag="s4", bufs=2
                                    )
                                    nc.vector.tensor_add(
                                        out=s4, in0=ph1.pop(hh), in1=s2
                                    )
                                    nc.tensor.matmul(
                                        pdb[hh],
                                        lhsT=ones_sb,
                                        rhs=s4,
                                        start=(kp == 3), stop=last,
                                    )
                        drain_quota()
                    # normalize this head pair: ctxT = pcs * (1/denominator)
                    for hh in range(2):
                        dinv = dipool.tile([P, QC], F32, name="dinv", tag="di")
                        nc.vector.reciprocal_approx_fast(out=dinv, in_=pdb[hh])
                        nc.vector.tensor_mul(
                            out=ct[:, h0 + hh, :], in0=pcs[hh], in1=dinv
                        )
                drain_wo(len(pending))  # anything left from previous chunk
                pending = make_wo_steps(b, qc, ct)

        drain_wo(len(pending))

    nc.compile()
    return nc


_CACHE = {}


def _get_graph():
    if "nc" not in _CACHE:
        _CACHE["nc"] = build_graph()
    return _CACHE["nc"]


def _host_prep(hidden_states, positions, Wq, Wk, Wv, Wo):
    """Transpose/cast/slice inputs per core. Returns list of 8 input dicts."""
    x2 = np.ascontiguousarray(hidden_states.reshape(T, HID).T).astype(BF16NP)

    pos = positions.astype(np.float32)                      # [B, S]
    half = HD // 2
    inv_freq = 1.0 / (ROPE_BASE ** (np.arange(half, dtype=np.float32) / half))
    ang = pos[:, :, None] * inv_freq[None, None, :]         # [B, S, 64]
    cos = np.cos(ang)
    sin = np.sin(ang)
    cosT = np.concatenate([cos[b].T for b in range(B)], axis=1)   # [64, T]
    sinT = np.concatenate([sin[b].T for b in range(B)], axis=1)
    cos2 = np.concatenate([cosT, cosT], axis=0).astype(BF16NP)    # [128, T]
    sin2 = np.concatenate([-sinT, sinT], axis=0).astype(BF16NP)

    r = np.arange(P)
    mneg = np.where(r[:, None] <= r[None, :], 0.0, -1e30).astype(np.float32)
    iden = np.eye(P, dtype=np.float32).astype(BF16NP)

    in_maps = []
    for c in range(NCORES):
        qs = slice(c * HPC * HD, (c + 1) * HPC * HD)
        ks = slice(c * HD, (c + 1) * HD)
        in_maps.append({
            "xT": x2,
            "wqT": np.ascontiguousarray(Wq[qs, :].T).astype(BF16NP),
            "wkT": np.ascontiguousarray(Wk[ks, :].T).astype(BF16NP),
            "wvT": np.ascontiguousarray(Wv[ks, :].T).astype(BF16NP),
            "woT": np.ascontiguousarray(Wo[:, qs].T).astype(BF16NP),
            "cos2": cos2,
            "sin2": sin2,
            "mneg": mneg,
            "iden": iden,
        })
    return in_maps


def kernel(hidden_states, positions, Wq, Wk, Wv, Wo):
    from concourse.bass_utils import run_bass_kernel_spmd

    trace = bool(os.environ.get("CLAUDE_KERNEL_TRACE"))
    if trace:
        _install_ntff_hook()

    nc = _get_graph()
    in_maps = _host_prep(
        np.asarray(hidden_states), np.asarray(positions),
        np.asarray(Wq), np.asarray(Wk), np.asarray(Wv), np.asarray(Wo),
    )
    res = run_bass_kernel_spmd(
        nc, in_maps, core_ids=list(range(NCORES)), trace=trace,
    )
    LAST["exec_time_ns"] = res.exec_time_ns
    LAST["profile_json"] = res.profile_json
    if res.instructions_and_trace is not None:
        LAST["trace_path"] = res.instructions_and_trace[1]

    acc = np.zeros((T, HID), np.float32)
    for c in range(NCORES):
        acc += res.results[c]["out"].astype(np.float32)
    return acc.reshape(B, S, HID)


# revision 16
# speedup vs baseline: 1.3714x; 1.1508x over previous
"""Trainium2 Bass kernel for GQA attention layer (RoPE + causal + GQA 32q/8kv).

Self-contained: hardcodes shapes from the problem spec.
  hidden_states [2, 2048, 4096] f32, positions [2, 2048] i32,
  Wq [4096, 4096], Wk [1024, 4096], Wv [1024, 4096], Wo [4096, 4096]  (all f32)
Sharding: tensor-parallel over heads across 8 cores. Core c gets kv head c and
q heads 4c..4c+3. Each core computes its partial Wo output; host sums partials.

v2 structure: scores stay [k,q]; PV computes ctxT = v.T @ probs directly
(N=512, v stationary reused over heads); softmax denominator via all-ones
matmul that lands pre-broadcast in PSUM; Wo matmuls interleaved into the next
chunk's attention inner loop so the PE never idles while scalar does exp.
"""

import math
import os
import sys
import types

import numpy as np
import ml_dtypes

BF16NP = ml_dtypes.bfloat16

# ---- problem constants (hardcoded per spec) ----
P = 128
B = 2
S = 2048            # tokens per batch
HID = 4096
NH, NKV, HD = 32, 8, 128
NCORES = 8
HPC = NH // NCORES  # q heads per core (4)
T = B * S
SCALE = 1.0 / math.sqrt(HD)
ROPE_BASE = 10000.0

LAST = {}           # exec_time_ns etc from the most recent run


def _install_ntff_hook():
    """Register the axon NTFF profiling hook (image's antenv lacks axon_hooks)."""
    if "antenv.axon_hooks" in sys.modules:
        return
    try:
        import antenv
        mod = types.ModuleType("antenv.axon_hooks")
        _box = [None]
        mod.set_axon_ntff_profile_hook = lambda h: _box.__setitem__(0, h)
        mod.get_axon_ntff_profile_hook = lambda: _box[0]
        sys.modules["antenv.axon_hooks"] = mod
        antenv.axon_hooks = mod
        from trn_agent_boot.trn_boot import _ntff_profile_via_ctypes
        mod.set_axon_ntff_profile_hook(
            _ntff_profile_via_ctypes("/opt/axon/libaxon_pjrt.so")
        )
    except Exception:
        pass


def build_graph(S_=S, HID_=HID, CH=512, QC=512):
    import concourse.bacc as bacc
    import concourse.mybir as mybir
    import concourse.tile as tile
    from contextlib import ExitStack

    BF = mybir.dt.bfloat16
    F32 = mybir.dt.float32
    Exp = mybir.ActivationFunctionType.Exp

    NKK = HID_ // P          # contraction tiles over hidden (32)
    NCH = S_ // CH           # proj chunks per batch (4)
    NQC = S_ // QC           # attention q chunks per batch (4)
    NST = QC // P            # q subtiles per chunk (4)
    NKT = S_ // P            # k tiles per batch (16)
    NVS = CH // P            # v row-subtiles per proj chunk (4)
    HOC = HID_ // 512        # output column chunks (8)
    OCW = 512                # output chunk width
    NHG = HPC // 2           # head-pair passes per q chunk (2)

    nc = bacc.Bacc(None)
    xT_h = nc.declare_dram_parameter("xT", [HID_, B * S_], BF, isOutput=False)
    wq_h = nc.declare_dram_parameter("wqT", [HID_, HPC * HD], BF, isOutput=False)
    wk_h = nc.declare_dram_parameter("wkT", [HID_, HD], BF, isOutput=False)
    wv_h = nc.declare_dram_parameter("wvT", [HID_, HD], BF, isOutput=False)
    wo_h = nc.declare_dram_parameter("woT", [HPC * HD, HID_], BF, isOutput=False)
    cos_h = nc.declare_dram_parameter("cos2", [P, B * S_], BF, isOutput=False)
    sin_h = nc.declare_dram_parameter("sin2", [P, B * S_], BF, isOutput=False)
    mneg_h = nc.declare_dram_parameter("mneg", [P, P], F32, isOutput=False)
    iden_h = nc.declare_dram_parameter("iden", [P, P], BF, isOutput=False)
    out_h = nc.declare_dram_parameter("out", [B * S_, HID_], BF, isOutput=True)

    xT_r = xT_h[:, :].rearrange("(ko ki) s -> ki ko s", ki=P)
    wq_r = wq_h[:, :].rearrange("(ko ki) d -> ki ko d", ki=P)
    wk_r = wk_h[:, :].rearrange("(ko ki) d -> ki ko d", ki=P)
    wv_r = wv_h[:, :].rearrange("(ko ki) d -> ki ko d", ki=P)
    wo_r = wo_h[:, :].rearrange("(oo oi) h -> oi oo h", oi=P)

    with tile.TileContext(nc) as tc, ExitStack() as ctx:
        wpool = ctx.enter_context(tc.tile_pool(name="wpool", bufs=1))
        qpool = ctx.enter_context(tc.tile_pool(name="qpool", bufs=1))
        kpool = ctx.enter_context(tc.tile_pool(name="kpool", bufs=1))
        vpool = ctx.enter_context(tc.tile_pool(name="vpool", bufs=1))
        xpool = ctx.enter_context(tc.tile_pool(name="xpool", bufs=3))
        cspool = ctx.enter_context(tc.tile_pool(name="cspool", bufs=2))
        rpool = ctx.enter_context(tc.tile_pool(name="rpool", bufs=3))
        vtpool = ctx.enter_context(tc.tile_pool(name="vtpool", bufs=2))
        pbpool = ctx.enter_context(tc.tile_pool(name="pbpool", bufs=6))
        dipool = ctx.enter_context(tc.tile_pool(name="dipool", bufs=4))
        ctpool = ctx.enter_context(tc.tile_pool(name="ctpool", bufs=2))
        opool = ctx.enter_context(tc.tile_pool(name="opool", bufs=3))

        psS = ctx.enter_context(tc.tile_pool(name="psS", bufs=2, space="PSUM"))
        psC = ctx.enter_context(tc.tile_pool(name="psC", bufs=2, space="PSUM"))
        psD = ctx.enter_context(tc.tile_pool(name="psD", bufs=2, space="PSUM"))
        psX = ctx.enter_context(tc.tile_pool(name="psX", bufs=2, space="PSUM"))

        # --- persistent weights / tables ---
        # Order DMAs so the first K-proj matmuls can start ASAP:
        # wk, then x chunk 0 + cos/sin come per-chunk below, then wv/wq.
        wk_sb = wpool.tile([P, NKK, HD], BF)
        nc.scalar.dma_start(out=wk_sb[:, 0:NKK // 2, :], in_=wk_r[:, 0:NKK // 2, :])
        nc.scalar.dma_start(out=wk_sb[:, NKK // 2:, :], in_=wk_r[:, NKK // 2:, :])
        wv_sb = wpool.tile([P, NKK, HD], BF)
        wq_sb = wpool.tile([P, NKK, HPC * HD], BF)
        wo_sb = wpool.tile([P, HPC, HID_], BF)
        mneg_sb = wpool.tile([P, P], F32)
        nc.scalar.dma_start(out=mneg_sb, in_=mneg_h[:, :])
        iden_sb = wpool.tile([P, P], BF)
        nc.scalar.dma_start(out=iden_sb, in_=iden_h[:, :])
        ones_sb = wpool.tile([P, P], BF)
        nc.vector.memset(ones_sb, 1.0)

        # Wo-step interleave queue: closures that emit a bit of the previous
        # chunk's Wo GEMM (4 matmuls + a psum drain copy [+ dma]).
        pending = []

        def drain_wo(n):
            for _ in range(min(n, len(pending))):
                pending.pop(0)()

        def make_wo_steps(b, qc, ct):
            """Build the list of Wo steps for q-chunk qc of batch b."""
            steps = []
            box = {}

            def step(hc, si4):
                def run():
                    if si4 == 0:
                        box[hc] = opool.tile([P, NST, OCW], BF, name="ob", tag="ob")
                    ob = box[hc]
                    po = psX.tile([P, OCW], F32, name="po", tag="px")
                    for ot in range(HPC):
                        nc.tensor.matmul(
                            po,
                            lhsT=ct[:, ot, si4 * P:(si4 + 1) * P],
                            rhs=wo_sb[:, ot, hc * OCW:(hc + 1) * OCW],
                            start=(ot == 0), stop=(ot == HPC - 1),
                        )
                    nc.vector.tensor_copy(out=ob[:, si4, :], in_=po)
                    if si4 == NST - 1:
                        orows = out_h[b * S_ + qc * QC: b * S_ + (qc + 1) * QC,
                                      hc * OCW:(hc + 1) * OCW]
                        nc.sync.dma_start(
                            out=orows.rearrange("(si p) h -> p si h", p=P),
                            in_=ob,
                        )
                return run

            for hc in range(HOC):
                for si4 in range(NST):
                    steps.append(step(hc, si4))
            return steps

        def rope(ps, dst, cs, sn):
            """Neox RoPE on [128 d, n] tile: rows 0:64 = first half of head dim."""
            qf = rpool.tile([P, CH], BF, tag="qf")
            nc.vector.tensor_copy(out=qf, in_=ps)
            qs = rpool.tile([P, CH], BF, tag="qs")
            nc.gpsimd.dma_start(out=qs[0:64, :], in_=qf[64:128, :])
            nc.gpsimd.dma_start(out=qs[64:128, :], in_=qf[0:64, :])
            nc.vector.tensor_mul(out=qf, in0=qf, in1=cs)
            nc.vector.tensor_mul(out=qs, in0=qs, in1=sn)
            nc.vector.tensor_add(out=dst, in0=qf, in1=qs)

        for b in range(B):
            # ---------- phase P: projections + RoPE ----------
            qT = qpool.tile([P, HPC, S_], BF)
            kT = kpool.tile([P, S_], BF)
            v = vpool.tile([P, NKT, P], BF)
            NKH = NKK // 2
            for t in range(NCH):
                c0 = b * S_ + t * CH
                c1 = c0 + CH
                xta = xpool.tile([P, NKH, CH], BF, tag="x")
                nc.sync.dma_start(out=xta, in_=xT_r[:, 0:NKH, c0:c1])
                xtb = xpool.tile([P, NKH, CH], BF, tag="x")
                nc.sync.dma_start(out=xtb, in_=xT_r[:, NKH:NKK, c0:c1])
                cs = cspool.tile([P, CH], BF, tag="cos")
                nc.gpsimd.dma_start(out=cs, in_=cos_h[:, c0:c1])
                sn = cspool.tile([P, CH], BF, tag="sin")
                nc.gpsimd.dma_start(out=sn, in_=sin_h[:, c0:c1])
                if b == 0 and t == 0:
                    # remaining weights, on the scalar DMA queue
                    nc.scalar.dma_start(out=wv_sb, in_=wv_r)
                    for _wi in range(4):
                        _lo, _hi = _wi * NKK // 4, (_wi + 1) * NKK // 4
                        nc.scalar.dma_start(
                            out=wq_sb[:, _lo:_hi, :], in_=wq_r[:, _lo:_hi, :]
                        )

                def xt(kk):
                    return xta[:, kk, :] if kk < NKH else xtb[:, kk - NKH, :]

                # K projection first (smallest weight dep)
                ps = psX.tile([P, CH], F32, tag="px")
                for kk in range(NKK):
                    nc.tensor.matmul(
                        ps, lhsT=wk_sb[:, kk, :], rhs=xt(kk),
                        start=(kk == 0), stop=(kk == NKK - 1),
                    )
                rope(ps, kT[:, t * CH:t * CH + CH], cs, sn)
                drain_wo(2)
                # V projection in vT orientation, then PE-transpose to [s, d]
                pv = psX.tile([P, CH], F32, tag="px")
                for kk in range(NKK):
                    nc.tensor.matmul(
                        pv, lhsT=wv_sb[:, kk, :], rhs=xt(kk),
                        start=(kk == 0), stop=(kk == NKK - 1),
                    )
                vt = vtpool.tile([P, CH], BF, tag="vt")
                nc.vector.tensor_copy(out=vt, in_=pv)
                for ss in range(NVS):
                    pq = psS.tile([P, P], BF, tag="s")
                    nc.tensor.transpose(pq, vt[:, ss * P:(ss + 1) * P], iden_sb)
                    nc.scalar.copy(out=v[:, t * NVS + ss, :], in_=pq)
                drain_wo(2)
                for g in range(HPC):
                    ps = psX.tile([P, CH], F32, tag="px")
                    for kk in range(NKK):
                        nc.tensor.matmul(
                            ps,
                            lhsT=wq_sb[:, kk, g * HD:(g + 1) * HD],
                            rhs=xt(kk),
                            start=(kk == 0), stop=(kk == NKK - 1),
                        )
                    rope(ps, qT[:, g, t * CH:t * CH + CH], cs, sn)
                    drain_wo(2)
                if b == 0 and t == 2:
                    for _wi in range(4):
                        _lo, _hi = _wi * HID_ // 4, (_wi + 1) * HID_ // 4
                        nc.scalar.dma_start(
                            out=wo_sb[:, :, _lo:_hi], in_=wo_r[:, :, _lo:_hi]
                        )

            # ---------- phase A: attention ----------
            for qc in range(NQC):
                nkt = (qc + 1) * NST
                ct = ctpool.tile([P, HPC, QC], BF, name="ct", tag="ct")
                # distribute leftover Wo work evenly over this chunk's iters
                niter = NHG * (nkt + 1)
                quota = len(pending) / max(niter, 1)
                acc = [0.0]

                def drain_quota():
                    acc[0] += quota
                    k = int(acc[0])
                    if k:
                        acc[0] -= k
                        drain_wo(k)

                for hg in range(NHG):
                    h0 = hg * 2
                    pcs = {}
                    pdb = {}
                    pbs = {}
                    ph1 = {}
                    for kt in range(nkt + 1):
                        if kt < nkt:
                            d = kt - qc * NST  # diag subtile index if >= 0
                            lo = d * P if d > 0 else 0
                            pss = {}
                            for hh in range(2):
                                h = h0 + hh
                                pt = psS.tile([P, QC], F32, name="pss", tag="s")
                                nc.tensor.matmul(
                                    pt[:, lo:QC],
                                    lhsT=kT[:, kt * P:(kt + 1) * P],
                                    rhs=qT[:, h, qc * QC + lo:(qc + 1) * QC],
                                    start=True, stop=True,
                                )
                                pss[hh] = pt
                            if d >= 0:
                                # mask strictly-upper part of the diagonal
                                # 128-subtile (keep k row <= q col)
                                for hh in range(2):
                                    nc.vector.tensor_add(
                                        out=pss[hh][:, d * P:(d + 1) * P],
                                        in0=pss[hh][:, d * P:(d + 1) * P],
                                        in1=mneg_sb,
                                    )
                            for hh in range(2):
                                pb = pbpool.tile([P, QC], BF, name="pb", tag="pb")
                                nc.scalar.activation(
                                    out=pb[:, lo:QC], in_=pss[hh][:, lo:QC],
                                    func=Exp, scale=SCALE,
                                )
                                if lo:
                                    nc.gpsimd.memset(pb[:, 0:lo], 0.0)
                                pbs[(kt, hh)] = pb
                        if kt > 0:
                            kp = kt - 1
                            first, last = (kp == 0), (kp == nkt - 1)
                            dp = kp - qc * NST
                            lop = dp * P if dp > 0 else 0
                            for hh in range(2):
                                pb = pbs[(kp, hh)]
                                if first:
                                    pcs[hh] = psC.tile(
                                        [P, QC], F32, name="pctx", tag="ctx"
                                    )
                                    pdb[hh] = psD.tile(
                                        [P, QC], F32, name="pdbc", tag="dbc"
                                    )
                                nc.tensor.matmul(
                                    pcs[hh][:, lop:QC],
                                    lhsT=v[:, kp, :],
                                    rhs=pb[:, lop:QC],
                                    start=first, stop=last,
                                )
                                # denominator: sum groups of 4 kt on gpsimd,
                                # one all-ones matmul per group (broadcast D)
                                if kp % 4 == 1:
                                    s2 = pbpool.tile(
                                        [P, QC], BF, name="s2", tag="s2", bufs=4
                                    )
                                    nc.vector.tensor_add(
                                        out=s2, in0=pbs.pop((kp - 1, hh)), in1=pb
                                    )
                                    del pbs[(kp, hh)]
                                    ph1[hh] = s2
                                elif kp % 4 == 3:
                                    s2 = pbpool.tile(
                                        [P, QC], BF, name="s2b", tag="s2b", bufs=2
                                    )
                                    nc.vector.tensor_add(
                                        out=s2, in0=pbs.pop((kp - 1, hh)), in1=pb
                                    )
                                    del pbs[(kp, hh)]
                                    s4 = pbpool.tile(
                                        [P, QC], BF, name="s4", tag="s4", bufs=2
                                    )
                                    nc.vector.tensor_add(
                                        out=s4, in0=ph1.pop(hh), in1=s2
                                    )
                                    nc.tensor.matmul(
                                        pdb[hh],
                                        lhsT=ones_sb,
                                        rhs=s4,
                                        start=(kp == 3), stop=last,
                                    )
                        drain_quota()
                    # normalize this head pair: ctxT = pcs * (1/denominator)
                    for hh in range(2):
                        dinv = dipool.tile([P, QC], F32, name="dinv", tag="di")
                        nc.vector.reciprocal_approx_fast(out=dinv, in_=pdb[hh])
                        nc.vector.tensor_mul(
                            out=ct[:, h0 + hh, :], in0=pcs[hh], in1=dinv
                        )
                drain_wo(len(pending))  # anything left from previous chunk
                pending = make_wo_steps(b, qc, ct)

        drain_wo(len(pending))

    nc.compile()
    return nc


_CACHE = {}


def _get_graph():
    if "nc" not in _CACHE:
        _CACHE["nc"] = build_graph()
    return _CACHE["nc"]


def _host_prep(hidden_states, positions, Wq, Wk, Wv, Wo):
    """Transpose/cast/slice inputs per core. Returns list of 8 input dicts."""
    x2 = np.ascontiguousarray(hidden_states.reshape(T, HID).T).astype(BF16NP)

    pos = positions.astype(np.float32)                      # [B, S]
    half = HD // 2
    inv_freq = 1.0 / (ROPE_BASE ** (np.arange(half, dtype=np.float32) / half))
    ang = pos[:, :, None] * inv_freq[None, None, :]         # [B, S, 64]
    cos = np.cos(ang)
    sin = np.sin(ang)
    cosT = np.concatenate([cos[b].T for b in range(B)], axis=1)   # [64, T]
    sinT = np.concatenate([sin[b].T for b in range(B)], axis=1)
    cos2 = np.concatenate([cosT, cosT], axis=0).astype(BF16NP)    # [128, T]
    sin2 = np.concatenate([-sinT, sinT], axis=0).astype(BF16NP)

    r = np.arange(P)
    mneg = np.where(r[:, None] <= r[None, :], 0.0, -1e30).astype(np.float32)
    iden = np.eye(P, dtype=np.float32).astype(BF16NP)

    in_maps = []
    for c in range(NCORES):
        qs = slice(c * HPC * HD, (c + 1) * HPC * HD)
        ks = slice(c * HD, (c + 1) * HD)
        in_maps.append({
            "xT": x2,
            "wqT": np.ascontiguousarray(Wq[qs, :].T).astype(BF16NP),
            "wkT": np.ascontiguousarray(Wk[ks, :].T).astype(BF16NP),
            "wvT": np.ascontiguousarray(Wv[ks, :].T).astype(BF16NP),
            "woT": np.ascontiguousarray(Wo[:, qs].T).astype(BF16NP),
            "cos2": cos2,
            "sin2": sin2,
            "mneg": mneg,
            "iden": iden,
        })
    return in_maps


def kernel(hidden_states, positions, Wq, Wk, Wv, Wo):
    from concourse.bass_utils import run_bass_kernel_spmd

    trace = bool(os.environ.get("CLAUDE_KERNEL_TRACE"))
    if trace:
        _install_ntff_hook()

    nc = _get_graph()
    in_maps = _host_prep(
        np.asarray(hidden_states), np.asarray(positions),
        np.asarray(Wq), np.asarray(Wk), np.asarray(Wv), np.asarray(Wo),
    )
    res = run_bass_kernel_spmd(
        nc, in_maps, core_ids=list(range(NCORES)), trace=trace,
    )
    LAST["exec_time_ns"] = res.exec_time_ns
    LAST["profile_json"] = res.profile_json
    if res.instructions_and_trace is not None:
        LAST["trace_path"] = res.instructions_and_trace[1]

    acc = np.zeros((T, HID), np.float32)
    for c in range(NCORES):
        acc += res.results[c]["out"].astype(np.float32)
    return acc.reshape(B, S, HID)
